# revision 3
# baseline (speedup 1.0000x reference)
"""Trainium2 Bass kernel for nn_Classifier — fp8 DoubleRow + PE pooling.

Reference computation (B=64, C=1280, H=W=7, A=40):
    p   = h_swish(mean(x, axis=(2,3)))            # [B, C]
    h   = h_swish(einsum("bc,acd->bad", p, W1) + b1)
    out = sigmoid(einsum("bac,ac->ba", h, W2) + b2)  # [B, A]

Sharding: 8 cores, each owns A/8 = 5 attribute heads.

vs the fp8 kernel2:
  - pooling moved OFF the DVE (TensorReduce runs at 1 elem/lane/cycle —
    ~33 us for 8 MB of x — and was the real bottleneck) onto the PE:
    x arrives as [(b,s) rows padded to 3328, C] fp8 and a constant 0/1
    selector [128, 2, 64] per 256-row chunk computes
    sums^T[b, c] = sum_r sel[r, b] * xP[r, c] as 13 DoubleRow matmuls
    per 512-wide n-chunk (~8.3k PE cycles total).
  - sums^T is evacuated to SBUF (bf16), PE-transposed back to [c, b]
    via the identity-ifmap trick, and Silu'd into the fp8 GEMM lhsT.
  - x is fp8 now (DMA 4.3 MB instead of 8 MB bf16).

Everything else (W1 fp8 DoubleRow GEMM, Silu evacuation, tanh-based
sigmoid, single ACT table set) matches kernel2.
"""

import sys

for _p in ("/opt/trn_rl_repo",):
    if _p not in sys.path:
        sys.path.insert(0, _p)

from contextlib import ExitStack

import numpy as np
import ml_dtypes

import concourse.bass as bass
import concourse.tile as tile
from concourse import bacc, mybir

# Problem constants (hardcoded per contract)
B = 64          # batch
C = 1280        # channels / features
S = 49          # spatial H*W
A = 40          # total attribute heads
NCORES = 8
AH = A // NCORES  # heads per core = 5
P = 128
KC = C // P       # 10 contraction chunks of 128
KC2 = KC // 2     # 5 DoubleRow chunks of 256
RS = B * S        # 3136 x rows
RP = 3328         # padded to 13 * 256
KX = RP // 256    # 13 DoubleRow pooling chunks
NS = [(0, 512), (512, 512), (1024, 256)]  # psum n-chunks of C=1280
G = 512.0         # fp8 weight scale (host folds in, removed at the end)

BF = mybir.dt.bfloat16
F32 = mybir.dt.float32
FP8 = mybir.dt.float8e4
AF = mybir.ActivationFunctionType
ALU = mybir.AluOpType
DR = mybir.MatmulPerfMode.DoubleRow

_NC_CACHE = {}


def build_nc(reps=1):
    """Per-core Bass program (same program on all 8 cores)."""
    nc = bacc.Bacc("TRN2", target_bir_lowering=False, name="attr_mlp8p")

    xP = nc.dram_tensor("xP", [RP, C], FP8, kind="ExternalInput")
    sel = nc.dram_tensor("sel", [P, KX * 2 * B], FP8, kind="ExternalInput")
    iden = nc.dram_tensor("iden", [B, B], BF, kind="ExternalInput")
    w1 = nc.dram_tensor("w1", [AH, C, C], FP8, kind="ExternalInput")
    b1 = nc.dram_tensor("b1", [AH * C], BF, kind="ExternalInput")
    w2b = nc.dram_tensor("w2b", [AH * B, C], BF, kind="ExternalInput")
    b2b = nc.dram_tensor("b2b", [AH * B], F32, kind="ExternalInput")
    out = nc.dram_tensor("out", [AH, B], F32, kind="ExternalOutput")

    with tile.TileContext(nc) as tc, ExitStack() as ctx:
        const = ctx.enter_context(tc.tile_pool(name="const", bufs=1))
        st = ctx.enter_context(tc.tile_pool(name="st", bufs=2))
        xp = ctx.enter_context(tc.tile_pool(name="xp", bufs=1))
        wg = ctx.enter_context(tc.tile_pool(name="wg", bufs=5))
        wp = ctx.enter_context(tc.tile_pool(name="wp", bufs=6))
        sp = ctx.enter_context(tc.tile_pool(name="sp", bufs=3))
        # PSUM budget (8 banks): pp 2x3, pq 1, pt 1
        pp = ctx.enter_context(tc.tile_pool(name="pp", bufs=2, space="PSUM"))
        pq = ctx.enter_context(tc.tile_pool(name="pq", bufs=1, space="PSUM"))
        pt = ctx.enter_context(tc.tile_pool(name="pt", bufs=1, space="PSUM"))

        # DoubleRow matmuls must write dst partitions 0-63 (ISA check
        # s3d3_mm_valid_dst_partition rejects tile_position col=64), so no
        # head pairing: heads run sequentially, pipelined via pp bufs=2.
        groups = [(h,) for h in range(AH)]

        # --- constants ---
        ones = const.tile([1, B], BF)
        nc.vector.memset(ones, 1.0)
        halfs = const.tile([P, 1], F32)  # 0.5 for the tanh->sigmoid affine
        nc.vector.memset(halfs, 0.5)
        # pooling selector + transpose identity lead the ACT ring (needed
        # before/with the first x chunks)
        sel_sb = const.tile([P, KX * 2, B], FP8)
        nc.scalar.dma_start(sel_sb, sel.rearrange("p (k b) -> p k b", b=B))
        id_sb = const.tile([B, B], BF)
        nc.scalar.dma_start(id_sb, iden[:, :])
        b1_sb = const.tile([1, AH * C], BF)
        nc.scalar.dma_start(b1_sb, b1[None, :])
        w2_g, b2_g = [], []
        for g, hs in enumerate(groups):
            pn = B * len(hs)
            r0 = hs[0] * B
            # scalar ring with the other small constants (~1 MB total);
            # the sync ring is reserved for x-then-W1 in priority order
            w2t = const.tile([pn, C], BF, tag=f"w2_{g}")
            nc.scalar.dma_start(w2t, w2b[r0:r0 + pn, :])
            b2t = const.tile([pn, 1], F32, tag=f"b2_{g}")
            nc.scalar.dma_start(b2t, b2b[r0:r0 + pn, None])
            w2_g.append(w2t)
            b2_g.append(b2t)

        for _rep in range(reps):
            # --- stage 1: pooling on the PE -> sums^T[b, c] in PSUM ---
            # x stays SBUF-resident (13 tiles, 33 KB/partition) so the three
            # 512-wide pooling passes reuse one PSUM bank.
            # x leads the sync ring: pooling gates the whole kernel, so x
            # must win the DMA bandwidth race against the W1 stream (same
            # FIFO ring => strict priority by issue order)
            xts = []
            for kk in range(KX):
                xt = xp.tile([P, 2, C], FP8, tag=f"xt{kk}")
                nc.sync.dma_start(
                    xt, xP[kk * 256:(kk + 1) * 256].rearrange(
                        "(two p) c -> p two c", two=2, p=P))
                xts.append(xt)
            sums_bf = st.tile([B, C], BF, tag="sums_bf")
            pT = st.tile([P, KC, B], FP8, tag="pT")
            for ni, (n0, nn) in enumerate(NS):
                pool_ps = pq.tile([B, 512], F32, tag="pool", name="pool")
                for kk in range(KX):
                    nc.tensor.matmul(
                        pool_ps[:, :nn], sel_sb[:, 2 * kk:2 * kk + 2, :],
                        xts[kk][:, :, n0:n0 + nn],
                        start=(kk == 0), stop=(kk == KX - 1),
                        perf_mode=DR, skip_group_check=True,
                    )
                nc.scalar.activation(
                    sums_bf[:, n0:n0 + nn], pool_ps[:, :nn], AF.Copy)
                # PE-transpose each ready 128-col tile to [c, b] and Silu
                # (scale 1/49) into the fp8 GEMM lhsT
                for k in range(n0 // P, (n0 + nn) // P):
                    tps = pt.tile([P, B], BF, tag="tps", name="tps")
                    nc.tensor.matmul(
                        tps, sums_bf[:, k * P:(k + 1) * P], id_sb,
                        is_transpose=True, start=True, stop=True,
                        skip_group_check=True,
                    )
                    nc.scalar.activation(pT[:, k, :], tps, AF.Silu,
                                         scale=1.0 / S)

            # --- stage 2: per-head-group fp8 DoubleRow GEMM + silu + dot(W2) ---
            for g, hs in enumerate(groups):
                pn = B * len(hs)
                nh = len(hs)
                last = g == len(groups) - 1
                pss = []
                for ni, (n0, nn) in enumerate(NS):
                    pst = pp.tile([P, nn], F32, tag=f"ps{ni}", name=f"ps{ni}")
                    pss.append(pst)

                a = hs[0]

                # one whole-head weight DMA (1.6 MB fp8); wg bufs=5 so all
                # heads prefetch behind the x stream. n-chunk-major matmul
                # order lets each n-chunk's evacuation overlap the next
                # n-chunk's matmuls (kills the end-of-kernel tail).
                w1g = wg.tile([P, KC, C], FP8, tag="w1g")
                src = w1[a].rearrange(
                    "(k two p) d -> p (k two) d", two=2, p=P)
                nc.sync.dma_start(w1g, src)
                for ni, (n0, nn) in enumerate(NS):
                    for k5 in range(KC2):
                        nc.tensor.matmul(
                            pss[ni][:B, :],
                            pT[:, 2 * k5:2 * k5 + 2, :],
                            w1g[:, 2 * k5:2 * k5 + 2, n0:n0 + nn],
                            start=(k5 == 0), stop=(k5 == KC2 - 1),
                            perf_mode=DR, skip_group_check=True,
                        )
                        if k5 == 0:
                            nc.tensor.matmul(
                                pss[ni][:B, :], ones,
                                b1_sb[:, a * C + n0:a * C + n0 + nn],
                                start=False, stop=False,
                                skip_group_check=True,
                            )
                # evacuation per n-chunk:
                #   h     = Silu(z/G)          [ACT, psum read, bf16 out]
                #   rpart = accum(h * w2)      [DVE stt 2x, f32 accum]
                rpart = st.tile([P, len(NS)], F32, tag="rpart")
                for ni, (n0, nn) in enumerate(NS):
                    zs = pss[ni][:pn]
                    h = sp.tile([P, 512], BF, tag="hbuf")
                    nc.scalar.activation(h[:pn, :nn], zs, AF.Silu, scale=1.0 / G)
                    hw2 = sp.tile([P, 512], BF, tag="hw2")
                    nc.vector.scalar_tensor_tensor(
                        hw2[:pn, :nn], h[:pn, :nn], 1.0,
                        w2_g[g][:, n0:n0 + nn], ALU.mult, ALU.mult,
                        accum_out=rpart[:pn, ni:ni + 1],
                    )
                rlog = st.tile([P, 1], F32, tag="rlog")
                nc.vector.reduce_sum(
                    rlog[:pn], rpart[:pn, :], axis=mybir.AxisListType.X
                )
                # sigmoid(v) = 0.5*tanh(v/2)+0.5; b2_g holds b2/2 (host).
                ost = st.tile([P, 1], F32, tag="ost")
                nc.scalar.activation(
                    ost[:pn], rlog[:pn], AF.Tanh, bias=b2_g[g], scale=0.5,
                )
                osb = st.tile([P, 1], F32, tag="osb")
                nc.vector.scalar_tensor_tensor(
                    osb[:pn], ost[:pn], 0.5, halfs[:pn], ALU.mult, ALU.add,
                )
                dst = out[hs[0]:hs[0] + len(hs), :].rearrange(
                    "h b -> (h b)")[:, None]
                if last:
                    # HWDGE latency beats SWDGE on the exposed final store
                    nc.sync.dma_start(dst, osb[:pn])
                else:
                    nc.gpsimd.dma_start(dst, osb[:pn])

    nc.compile()
    return nc


def get_nc(reps=1):
    if reps not in _NC_CACHE:
        _NC_CACHE[reps] = build_nc(reps)
    return _NC_CACHE[reps]


def make_in_maps(x, W1, b1, W2, b2):
    bf = ml_dtypes.bfloat16
    e4 = ml_dtypes.float8_e4m3
    x = np.asarray(x, dtype=np.float32)
    W1 = np.asarray(W1, dtype=np.float32)
    b1 = np.asarray(b1, dtype=np.float32)
    W2 = np.asarray(W2, dtype=np.float32)
    b2 = np.asarray(b2, dtype=np.float32)

    # x as [(b, s) rows, C], zero-padded to RP rows, fp8
    xPm = np.zeros((RP, C), dtype=e4)
    xPm[:RS] = x.reshape(B, C, S).transpose(0, 2, 1).reshape(RS, C).astype(e4)

    # selector: sel[p, kk*2+i, b] = 1 if row kk*256+i*128+p belongs to batch b
    rows = np.arange(RP)
    owner = np.where(rows < RS, rows // S, -1)  # batch index per row, -1 pad
    selm = np.zeros((P, KX * 2, B), dtype=np.float32)
    for kk in range(KX):
        for i in range(2):
            r = kk * 256 + i * 128 + np.arange(P)
            o = owner[r]
            valid = o >= 0
            selm[np.arange(P)[valid], kk * 2 + i, o[valid]] = 1.0
    selm = selm.reshape(P, KX * 2 * B).astype(e4)

    idenm = np.eye(B, dtype=np.float32).astype(bf)

    in_maps = []
    for core in range(NCORES):
        a0 = core * AH
        w2s = W2[a0:a0 + AH]  # [AH, C]
        in_maps.append({
            "xP": xPm,
            "sel": selm,
            "iden": idenm,
            "w1": np.ascontiguousarray(W1[a0:a0 + AH] * G).astype(e4),
            "b1": np.ascontiguousarray(b1[a0:a0 + AH] * G).reshape(AH * C).astype(bf),
            "w2b": np.ascontiguousarray(
                np.broadcast_to(w2s[:, None, :], (AH, B, C)).reshape(AH * B, C)
            ).astype(bf),
            # halved: the tanh-based sigmoid computes tanh(logit/2 + b2/2)
            "b2b": np.ascontiguousarray(
                np.broadcast_to(b2[a0:a0 + AH, None] * 0.5, (AH, B)).reshape(AH * B)
            ).astype(np.float32),
        })
    return in_maps


def kernel(x, W1, b1, W2, b2, _trace=False, _tmpdir=None):
    from concourse.bass_utils import run_bass_kernel_spmd

    nc = get_nc()
    in_maps = make_in_maps(x, W1, b1, W2, b2)
    res = run_bass_kernel_spmd(
        nc, in_maps, core_ids=list(range(NCORES)),
        trace=_trace, tmpdir=_tmpdir,
    )
    outs = [np.asarray(res.results[c]["out"], dtype=np.float32).T
            for c in range(NCORES)]  # each [B, AH]
    full = np.concatenate(outs, axis=1)  # [B, A]
    if _trace:
        return full, res
    return full
